# revision 1
# baseline (speedup 1.0000x reference)
import os
import sys

import numpy as np

for _p in ("/opt/trn_rl_repo", "/root/.axon_site/_ro/trn_rl_repo"):
    if os.path.isdir(_p) and _p not in sys.path:
        sys.path.insert(0, _p)

# Problem constants (nn_CRF: feats [B,S,T] f32, masks [B,S] ones, transitions [T,T])
B, S, T = 512, 1024, 64
NC = 8            # cores
BL = B // NC      # 64 batches per core
NGRP = 2          # independent batch groups per core (pipelining)
BG = BL // NGRP   # 32
F = 6.0           # upper bound on |feat|; g = exp(feat - F) <= 1
KR = 12           # renormalize every KR g-applications
DBLK = 16         # time steps per DMA block
NBLK = S // DBLK  # 64
NEG = -10000.0

_CACHE = {}


def _build_bass(repeats=None):
    import concourse.bacc as bacc
    import concourse.mybir as mybir
    from concourse.tile import TileContext
    from concourse import bass_isa
    import contextlib

    f32 = mybir.dt.float32
    bf16 = mybir.dt.bfloat16
    Ln = mybir.ActivationFunctionType.Ln

    nc = bacc.Bacc()
    # g arranged host-side as [NBLK, T, DBLK, BL] so each DMA block is
    # contiguous per partition (DBLK*BL*2B = 2KB lines).
    g_in = nc.dram_tensor("g", [NBLK, T, DBLK, BL], bf16, kind="ExternalInput")
    # lhsT for the step matmul: [k, j] = exp(transitions[j, k])
    et_in = nc.dram_tensor("eaug", [T, T], bf16, kind="ExternalInput")
    xout = nc.dram_tensor("xout", [T, BL], bf16, kind="ExternalOutput")
    aux = nc.dram_tensor("aux", [2, BL], f32, kind="ExternalOutput")

    NX = 4      # X state rotation slots (gives gpsimd slack to read old X)
    LAG = 4     # renorm scale measured at app b is folded into g at app b+LAG

    with TileContext(nc) as tc:
        with tc.tile_pool(name="const", bufs=1) as cpool, \
             tc.tile_pool(name="gp", bufs=3) as gpool, \
             tc.tile_pool(name="state", bufs=1) as xpool, \
             tc.tile_pool(name="ps", bufs=1, space="PSUM") as pspool, \
             tc.tile_pool(name="misc", bufs=2) as mpool:
            et_stage = cpool.tile([T, T], bf16)
            nc.sync.dma_start(et_stage, et_in[:, :])
            et = cpool.tile([T, T], bf16)
            # copy via DVE so matmuls depend only on the DVE semaphore
            nc.vector.tensor_copy(et, et_stage)
            loop_cm = tc.For_i(0, repeats, 1) if repeats else contextlib.nullcontext()
            with loop_cm:
                xs, crows, pss, sbs, rbss, gs2s = [], [], [], [], [], []
                for gi in range(NGRP):
                    rot = []
                    for sl_i in range(NX):
                        x_t = xpool.tile([T, BG], bf16, tag=f"x{gi}_{sl_i}")
                        rot.append(x_t)
                    xs.append(rot)
                    cr = xpool.tile([1, BG], f32, tag=f"c{gi}")
                    nc.vector.memset(cr, 0.0)
                    crows.append(cr)
                    ps_t = pspool.tile([T, BG], f32, tag=f"ps{gi}")
                    pss.append(ps_t)
                    sb_t = xpool.tile([T, BG], f32, tag=f"sb{gi}")
                    sbs.append(sb_t)
                    rbs_t = xpool.tile([T, BG], f32, tag=f"rbs{gi}")
                    rbss.append(rbs_t)
                    gs2_t = xpool.tile([T, BG], bf16, tag=f"gs2{gi}")
                    gs2s.append(gs2_t)
                # pending[gi] = app index whose g-slice must be scaled by rbss
                pending = [None] * NGRP
                sb_last = [None] * NGRP
                app = 0
                for blk in range(NBLK):
                    gt = gpool.tile([T, DBLK, BL], bf16, tag="g")
                    nc.sync.dma_start(gt, g_in[blk])
                    for t in range(DBLK):
                        for gi in range(NGRP):
                            gsl = gt[:, t, gi * BG:(gi + 1) * BG]
                            if app == 0:
                                nc.vector.tensor_copy(xs[gi][0], gsl)
                                continue
                            xprev = xs[gi][(app - 1) % NX]
                            xcur = xs[gi][app % NX]
                            ps = pss[gi]
                            nc.tensor.matmul(ps, et, xprev, start=True, stop=True)
                            if pending[gi] == app:
                                # fold the pending 1/s renorm into this g slice
                                # (off the critical chain: all-SBUF DVE ops)
                                nc.vector.tensor_mul(gs2s[gi], gsl, rbss[gi])
                                gsl = gs2s[gi]
                                pending[gi] = None
                            nc.vector.tensor_mul(xcur, gsl, ps)
                            is_tap = (app % KR == KR - 1 and app + LAG <= S - 2)
                            if is_tap or app == S - 2:
                                # partition sum of X_app, broadcast to all
                                # partitions; gpsimd runs off the chain
                                nc.gpsimd.partition_all_reduce(
                                    sbs[gi], xcur, T, bass_isa.ReduceOp.add)
                                ls = mpool.tile([1, BG], f32, tag=f"l{gi}")
                                nc.scalar.activation(ls, sbs[gi][0:1, :], Ln)
                                if is_tap:
                                    nc.vector.reciprocal(rbss[gi], sbs[gi])
                                    nc.vector.tensor_add(crows[gi], crows[gi], ls)
                                    pending[gi] = app + LAG
                                if app == S - 2:
                                    sb_last[gi] = ls
                        app += 1
                for gi in range(NGRP):
                    cs = slice(gi * BG, (gi + 1) * BG)
                    nc.sync.dma_start(xout[:, cs], xs[gi][(S - 1) % NX])
                    nc.sync.dma_start(aux[0:1, cs], crows[gi])
                    nc.sync.dma_start(aux[1:2, cs], sb_last[gi])
    nc.finalize()
    return nc


def _numpy_ref(feats, masks, transitions):
    # Exact log-domain fallback (only used if masks are not all ones).
    alpha = feats[:, 0].astype(np.float64)
    tr = transitions.astype(np.float64)
    for i in range(1, feats.shape[1]):
        sc = alpha[:, None, :] + tr[None] + feats[:, i, :, None].astype(np.float64)
        m = sc.max(axis=2, keepdims=True)
        new = (m[:, :, 0] + np.log(np.exp(sc - m).sum(axis=2)))
        mask = masks[:, i, None].astype(np.float64)
        alpha = new * mask + alpha * (1.0 - mask)
    return alpha.astype(np.float32)


def kernel(feats, masks, transitions):
    feats = np.asarray(feats, dtype=np.float32)
    masks = np.asarray(masks, dtype=np.float32)
    transitions = np.asarray(transitions, dtype=np.float32)
    if not np.all(masks == 1.0):
        return _numpy_ref(feats, masks, transitions)

    from concourse import bass_utils

    if "nc" not in _CACHE:
        _CACHE["nc"] = _build_bass()
    nc = _CACHE["nc"]

    E = np.exp(transitions)                      # [j,k]; row/col 0 -> 0
    # overflow-safety: per-step growth bound must fit f32 over a KR+4 window
    grow = float(np.log(E.sum(axis=1)).max())
    assert (KR + 4) * max(grow, 0.0) < 85.0, grow
    eaug = np.ascontiguousarray(E.T)
    # g[b,s,j] = exp(feats - F) -> per-core [NBLK, T, DBLK, BL] bf16
    g = np.exp(feats - F)
    g = g.reshape(NC, BL, NBLK, DBLK, T).transpose(0, 2, 4, 3, 1)
    g = np.ascontiguousarray(g, dtype=np.float32)
    import ml_dtypes
    g16 = g.astype(ml_dtypes.bfloat16)

    in_maps = [{"g": g16[c], "eaug": eaug.astype(ml_dtypes.bfloat16)} for c in range(NC)]
    trace = bool(os.environ.get("CRF_TRACE"))
    import time as _time
    _t0 = _time.time()
    res = bass_utils.run_bass_kernel_spmd(
        nc, in_maps, core_ids=list(range(NC)), trace=trace)
    _CACHE.setdefault("t_run", []).append(_time.time() - _t0)
    _CACHE["last_res"] = res

    alpha = np.empty((B, T), np.float32)
    for c in range(NC):
        X = res.results[c]["xout"].astype(np.float64)    # [T, BL]
        crow = res.results[c]["aux"][0].astype(np.float64)   # [BL]
        lsl = res.results[c]["aux"][1].astype(np.float64)    # ln sum_k X_{S-2}
        a = np.log(np.maximum(X.T, 1e-300)) + (S * F + crow)[:, None]
        a[:, 0] = (feats[c * BL:(c + 1) * BL, S - 1, 0] + NEG
                   + lsl + (S - 1) * F + crow)
        alpha[c * BL:(c + 1) * BL] = a.astype(np.float32)
    return alpha

